# revision 2
# baseline (speedup 1.0000x reference)
"""Trainium2 Bass kernel: logistic CP tensor factor model.

Computes sigmoid(einsum("id,jd,kd->ijk", W, V, U)) for
I=1024, J=512, K=256, D=64 (float32), sharded over 8 NeuronCores
along I (128 rows each, embarrassingly parallel, no communication).

Per-core formulation: out[i, (j,k)] = sigmoid( W_loc @ G.T ) where
G[(j,k), d] = V[j,d] * U[k,d] is the Khatri-Rao product, identical on
every core. The PE computes out chunks with W_loc.T as the (near)
stationary operand, so every output partition (one i) writes fully
contiguous DRAM runs -- the kernel is output-write-bandwidth bound
(64 MiB/core, ~358 GB/s/core HBM limit -> ~187 us floor).

Pipeline per 32-j group (16 groups/core):
  DVE:  G.T chunk [128p(d x 2 j-halves), 16j x 256k] = V-bcast * U-bcast (bf16 out)
  PE:   16 matmuls [128,512] (lhsT = zero-padded W.T halves, bf16)
  ACT:  4 x sigmoid [128, 2048] PSUM->SBUF (f32)
  DMA:  1 x 4 MiB stage tile -> DRAM (contiguous 32 KiB per partition)
"""

import numpy as np
import ml_dtypes

I, J, K, D = 1024, 512, 256, 64
N_CORES = 8
IL = I // N_CORES  # 128 rows of W per core
JK = J * K  # 131072
N_GROUPS = 16  # groups of 32 j's
GJ = J // N_GROUPS  # 32 j's per group
GCOLS = GJ * K  # 8192 output cols per group

_compiled = None  # (nc, run) cache across calls within one process


def _build():
    import concourse.mybir as mybir
    import concourse.tile as tile
    from concourse import bacc

    f32 = mybir.dt.float32
    bf16 = mybir.dt.bfloat16

    nc = bacc.Bacc("TRN2", target_bir_lowering=False, debug=False,
                   num_devices=N_CORES)

    # WTZ: [128, 256] bf16; cols 0:128 = [W_loc.T; 0], cols 128:256 = [0; W_loc.T]
    wtz_d = nc.dram_tensor("WTZ", [128, 2 * IL], bf16, kind="ExternalInput")
    # VT2: [128, 512] f32; rows 0:64 = V.T, rows 64:128 = V.T shifted left 16 cols
    vt2_d = nc.dram_tensor("VT2", [128, J], f32, kind="ExternalInput")
    # UT2: [128, 256] f32; rows 0:64 = U.T, rows 64:128 = U.T
    ut2_d = nc.dram_tensor("UT2", [128, K], f32, kind="ExternalInput")
    out_d = nc.dram_tensor("out", [IL, JK], f32, kind="ExternalOutput")

    sig = mybir.ActivationFunctionType.Sigmoid

    with tile.TileContext(nc) as tc:
        with (
            tc.tile_pool(name="const", bufs=1) as cpool,
            tc.tile_pool(name="gt", bufs=3) as gpool,
            tc.tile_pool(name="stage", bufs=3) as spool,
            tc.tile_pool(name="psum", bufs=2, space="PSUM") as ppool,
        ):
            wtz = cpool.tile([128, 2 * IL], bf16, tag="wtz")
            vt2 = cpool.tile([128, J], f32, tag="vt2")
            ut2 = cpool.tile([128, K], f32, tag="ut2")
            nc.sync.dma_start(wtz[:, :], wtz_d[:, :])
            nc.sync.dma_start(vt2[:, :], vt2_d[:, :])
            nc.sync.dma_start(ut2[:, :], ut2_d[:, :])

            for g in range(N_GROUPS):
                j0 = g * GJ
                # G.T chunk: gt[p, jloc*K + k] = V[j0 + 16h + jloc, d] * U[k, d]
                # where p = 64h + d (h = partition half).
                gt = gpool.tile([128, GJ // 2 * K], bf16, tag="gt")
                nc.vector.tensor_mul(
                    gt[:, :].rearrange("p (j k) -> p j k", j=GJ // 2),
                    vt2[:, j0:j0 + GJ // 2].unsqueeze(2)
                        .broadcast_to([128, GJ // 2, K]),
                    ut2[:, :].unsqueeze(1).broadcast_to([128, GJ // 2, K]),
                )
                st = spool.tile([128, GCOLS], f32, tag="st")
                for q in range(4):  # psum tile = 2048 output cols
                    h = q // 2
                    pt = ppool.tile([128, 2048], f32, tag="pt")
                    for s in range(4):  # one matmul = 512 output cols
                        wm = (q % 2) * 4 + s
                        nc.tensor.matmul(
                            pt[:, 512 * s:512 * (s + 1)],
                            lhsT=wtz[:, 128 * h:128 * (h + 1)],
                            rhs=gt[:, 512 * wm:512 * (wm + 1)],
                            start=True, stop=True,
                        )
                    nc.scalar.activation(st[:, 2048 * q:2048 * (q + 1)],
                                         pt[:, :], sig)
                nc.sync.dma_start(out_d[:, GCOLS * g:GCOLS * (g + 1)], st[:, :])

    nc.compile()
    return nc


def _prep_inputs(W, V, U):
    """Host-side layout prep (cheap: inputs are <0.5 MB total)."""
    W = np.asarray(W, dtype=np.float32)
    V = np.asarray(V, dtype=np.float32)
    U = np.asarray(U, dtype=np.float32)
    VT = V.T  # [64, 512]
    # upper half: shifted left by 16 j's, zero padded
    VT2 = np.zeros((128, J), dtype=np.float32)
    VT2[:64] = VT
    VT2[64:, :J - GJ // 2] = VT[:, GJ // 2:]
    UT2 = np.concatenate([U.T, U.T], axis=0).astype(np.float32)  # [128, 256]
    in_maps = []
    for c in range(N_CORES):
        WT = W[c * IL:(c + 1) * IL].T.astype(ml_dtypes.bfloat16)  # [64, 128]
        WTZ = np.zeros((128, 2 * IL), dtype=ml_dtypes.bfloat16)
        WTZ[:64, :IL] = WT
        WTZ[64:, IL:] = WT
        in_maps.append({"WTZ": WTZ, "VT2": VT2, "UT2": UT2})
    return in_maps


def _get_nc():
    global _compiled
    if _compiled is None:
        _compiled = _build()
    return _compiled


def _run(inputs, profile_dir=None):
    """Returns (full_output, exec_time_ns or None)."""
    nc = _get_nc()
    in_maps = _prep_inputs(inputs["W"], inputs["V"], inputs["U"])
    exec_ns = None
    if profile_dir is not None:
        from bass_prof import profile_run  # test-only sibling module
        results, exec_ns = profile_run(nc, in_maps, N_CORES, profile_dir)
    else:
        from concourse.bass_utils import run_bass_kernel_spmd
        try:
            results = run_bass_kernel_spmd(nc, in_maps, list(range(N_CORES))).results
        except Exception:
            # rare transient device-unrecoverable; the runtime resets on
            # reconnect, so one retry is usually enough
            import time
            time.sleep(5)
            results = run_bass_kernel_spmd(nc, in_maps, list(range(N_CORES))).results
    out = np.concatenate(
        [results[c]["out"].reshape(IL, J, K) for c in range(N_CORES)], axis=0
    )
    return out, exec_ns


def kernel(W, V, U):
    out, _ = _run({"W": W, "V": V, "U": U})
    return out


# revision 4
# speedup vs baseline: 1.0572x; 1.0572x over previous
"""Trainium2 Bass kernel: logistic CP tensor factor model.

Computes sigmoid(einsum("id,jd,kd->ijk", W, V, U)) for
I=1024, J=512, K=256, D=64 (float32), sharded over 8 NeuronCores
along I (128 rows each, embarrassingly parallel, no communication).

Per-core formulation: out[i, (j,k)] = sigmoid( W_loc @ G.T ) where
G[(j,k), d] = V[j,d] * U[k,d] is the Khatri-Rao product, identical on
every core. The PE computes out chunks with W_loc.T as the (near)
stationary operand, so every output partition (one i) writes fully
contiguous DRAM runs -- the kernel is output-write-bandwidth bound
(64 MiB/core, ~358 GB/s/core HBM limit -> ~187 us floor).

Pipeline per 32-j group (16 groups/core):
  DVE:  G.T chunk [128p(d x 2 j-halves), 16j x 256k] = V-bcast * U-bcast (bf16 out)
  PE:   16 matmuls [128,512] (lhsT = zero-padded W.T halves, bf16)
  ACT:  4 x sigmoid [128, 2048] PSUM->SBUF (f32)
  DMA:  1 x 4 MiB stage tile -> DRAM (contiguous 32 KiB per partition)
"""

import os

import numpy as np
import ml_dtypes

I, J, K, D = 1024, 512, 256, 64
N_CORES = 8
IL = I // N_CORES  # 128 rows of W per core
JK = J * K  # 131072
N_GROUPS = 16  # groups of 32 j's
GJ = J // N_GROUPS  # 32 j's per group
GCOLS = GJ * K  # 8192 output cols per group

_compiled = None  # (nc, run) cache across calls within one process


def _build():
    import concourse.mybir as mybir
    import concourse.tile as tile
    from concourse import bacc

    f32 = mybir.dt.float32
    bf16 = mybir.dt.bfloat16

    nc = bacc.Bacc("TRN2", target_bir_lowering=False, debug=False,
                   num_devices=N_CORES)

    # WTZ: [128, 256] bf16; cols 0:128 = [W_loc.T; 0], cols 128:256 = [0; W_loc.T]
    wtz_d = nc.dram_tensor("WTZ", [128, 2 * IL], bf16, kind="ExternalInput")
    # VT2: [128, 512] f32; rows 0:64 = V.T, rows 64:128 = V.T shifted left 16 cols
    vt2_d = nc.dram_tensor("VT2", [128, J], f32, kind="ExternalInput")
    # UT2: [128, 256] f32; rows 0:64 = U.T, rows 64:128 = U.T
    ut2_d = nc.dram_tensor("UT2", [128, K], f32, kind="ExternalInput")
    out_d = nc.dram_tensor("out", [IL, JK], f32, kind="ExternalOutput")

    sig = mybir.ActivationFunctionType.Sigmoid

    with tile.TileContext(nc) as tc:
        with (
            tc.tile_pool(name="const", bufs=1) as cpool,
            tc.tile_pool(name="gt", bufs=3) as gpool,
            tc.tile_pool(name="stage", bufs=3) as spool,
            tc.tile_pool(name="psum", bufs=2, space="PSUM") as ppool,
        ):
            wtz = cpool.tile([128, 2 * IL], bf16, tag="wtz")
            vt2 = cpool.tile([128, J], f32, tag="vt2")
            ut2 = cpool.tile([128, K], f32, tag="ut2")
            nc.sync.dma_start(wtz[:, :], wtz_d[:, :])
            nc.sync.dma_start(vt2[:, :], vt2_d[:, :])
            nc.sync.dma_start(ut2[:, :], ut2_d[:, :])

            for g in range(N_GROUPS):
                j0 = g * GJ
                # G.T chunk: gt[p, jloc*K + k] = V[j0 + 16h + jloc, d] * U[k, d]
                # where p = 64h + d (h = partition half).
                gt = gpool.tile([128, GJ // 2 * K], bf16, tag="gt")
                nc.vector.tensor_mul(
                    gt[:, :].rearrange("p (j k) -> p j k", j=GJ // 2),
                    vt2[:, j0:j0 + GJ // 2].unsqueeze(2)
                        .broadcast_to([128, GJ // 2, K]),
                    ut2[:, :].unsqueeze(1).broadcast_to([128, GJ // 2, K]),
                )
                st = spool.tile([128, GCOLS], f32, tag="st")
                for q in range(4):  # psum tile = 2048 output cols
                    h = q // 2
                    pt = ppool.tile([128, 2048], f32, tag="pt")
                    for s in range(4):  # one matmul = 512 output cols
                        wm = (q % 2) * 4 + s
                        nc.tensor.matmul(
                            pt[:, 512 * s:512 * (s + 1)],
                            lhsT=wtz[:, 128 * h:128 * (h + 1)],
                            rhs=gt[:, 512 * wm:512 * (wm + 1)],
                            start=True, stop=True,
                        )
                    nc.scalar.activation(st[:, 2048 * q:2048 * (q + 1)],
                                         pt[:, :], sig)
                nc.sync.dma_start(out_d[:, GCOLS * g:GCOLS * (g + 1)], st[:, :])

    nc.compile()
    return nc


def _prep_inputs(W, V, U):
    """Host-side layout prep (cheap: inputs are <0.5 MB total)."""
    W = np.asarray(W, dtype=np.float32)
    V = np.asarray(V, dtype=np.float32)
    U = np.asarray(U, dtype=np.float32)
    VT = V.T  # [64, 512]
    # upper half: shifted left by 16 j's, zero padded
    VT2 = np.zeros((128, J), dtype=np.float32)
    VT2[:64] = VT
    VT2[64:, :J - GJ // 2] = VT[:, GJ // 2:]
    UT2 = np.concatenate([U.T, U.T], axis=0).astype(np.float32)  # [128, 256]
    in_maps = []
    for c in range(N_CORES):
        WT = W[c * IL:(c + 1) * IL].T.astype(ml_dtypes.bfloat16)  # [64, 128]
        WTZ = np.zeros((128, 2 * IL), dtype=ml_dtypes.bfloat16)
        WTZ[:64, :IL] = WT
        WTZ[64:, IL:] = WT
        in_maps.append({"WTZ": WTZ, "VT2": VT2, "UT2": UT2})
    return in_maps


def _get_nc():
    global _compiled
    if _compiled is None:
        _compiled = _build()
    return _compiled


def _run(inputs, profile_dir=None):
    """Returns (full_output, exec_time_ns or None)."""
    nc = _get_nc()
    in_maps = _prep_inputs(inputs["W"], inputs["V"], inputs["U"])
    exec_ns = None
    if profile_dir is not None:
        from bass_prof import profile_run  # test-only sibling module
        results, exec_ns = profile_run(nc, in_maps, N_CORES, profile_dir)
    else:
        from concourse.bass_utils import run_bass_kernel_spmd
        results = run_bass_kernel_spmd(nc, in_maps, list(range(N_CORES))).results
    out = np.concatenate(
        [results[c]["out"].reshape(IL, J, K) for c in range(N_CORES)], axis=0
    )
    return out, exec_ns


def _run_in_subprocess(W, V, U):
    """Execute in a fresh interpreter: a wedged device (rare transient
    NRT_EXEC_UNIT_UNRECOVERABLE) recovers on a fresh PJRT connection but
    not within the poisoned process."""
    import subprocess
    import sys
    import tempfile

    with tempfile.TemporaryDirectory() as td:
        inp = os.path.join(td, "in.npz")
        outp = os.path.join(td, "out.npy")
        np.savez(inp, W=np.asarray(W, np.float32), V=np.asarray(V, np.float32),
                 U=np.asarray(U, np.float32))
        subprocess.run(
            [sys.executable, os.path.abspath(__file__), "--selfrun", inp, outp],
            check=True, timeout=1800,
        )
        return np.load(outp)


def kernel(W, V, U):
    import time
    inputs = {"W": W, "V": V, "U": U}
    try:
        out, _ = _run(inputs)
        return out
    except Exception:
        time.sleep(10)
    try:  # in-process retry (covers transient RPC errors)
        out, _ = _run(inputs)
        return out
    except Exception:
        time.sleep(10)
    # fresh-process retry (covers device-unrecoverable poisoning the client)
    try:
        return _run_in_subprocess(W, V, U)
    except Exception:
        time.sleep(30)
        return _run_in_subprocess(W, V, U)


if __name__ == "__main__":
    import sys
    if len(sys.argv) == 4 and sys.argv[1] == "--selfrun":
        data = np.load(sys.argv[2])
        result, _ = _run({"W": data["W"], "V": data["V"], "U": data["U"]})
        np.save(sys.argv[3], result)
